# revision 1
# baseline (speedup 1.0000x reference)
"""GAT layer on trn2 v2: edge-parallel, aligned 128-node windows, 4 s-chunks
gathered via dma_gather (int16), PSUM one-hot aggregation, a_dst via
replicate-matmul + is_equal + accumulating expand-matmul.
"""

import numpy as np

import concourse.bacc as bacc
import concourse.bass as bass
import concourse.mybir as mybir
import concourse.tile as tile
from concourse.bass_utils import run_bass_kernel_spmd

AF = mybir.ActivationFunctionType
ALU = mybir.AluOpType
DT = mybir.dt

P = 128


# ---------------------------------------------------------------- host prep

def host_prep(x, ei, ea, n_cores, heads, head_dim, wb=5):
    """Sort edges by dest; aligned 128-node windows; 4 equal s-chunks.

    Slot layouts (per core, all uniform shapes):
      gather stream: tau = q*(WPC*TC_) + w*TC_ + k   (chunk-major)
      compute order: window-major (w, q, k)
    wb: windows per batch-set (gather batch = wb*TC_ tiles per chunk).
    """
    N = x.shape[0]
    E = ei.shape[1]
    edim = ea.shape[1]
    NPAD = ((N + P - 1) // P) * P
    CH = NPAD // 4  # chunk rows (<= 32767 required)
    assert CH <= 32767

    s = ei[0].astype(np.int64)
    d = ei[1].astype(np.int64)
    perm = np.argsort(d, kind="stable")
    s_s = s[perm].astype(np.int64)
    d_s = d[perm].astype(np.int64)
    ea_s = ea[perm]

    cuts_e = [0]
    node_lo = [0]
    for k in range(1, n_cores):
        t = k * E // n_cores
        node = int(d_s[min(t, E - 1)])
        node = (node // P) * P  # align core boundaries to 128 too
        cut = int(np.searchsorted(d_s, node, side="left"))
        cuts_e.append(cut)
        node_lo.append(node)
    cuts_e.append(E)
    node_hi = node_lo[1:] + [N]

    WPC = max(
        (node_hi[k] - node_lo[k] + P - 1) // P for k in range(n_cores)
    )
    # pad WPC to multiple of wb
    WPC = ((WPC + wb - 1) // wb) * wb

    # per (core, window, chunk) edge counts -> uniform tiles per cell
    TC_ = 0
    groups = []
    for k in range(n_cores):
        e0, e1 = cuts_e[k], cuts_e[k + 1]
        nlo = node_lo[k]
        w_of = (d_s[e0:e1] - nlo) // P
        q_of = s_s[e0:e1] // CH
        order = np.lexsort((q_of, w_of))  # stable by (w, q)
        groups.append((e0, order, w_of, q_of))
        wq = w_of * (4) + q_of
        if len(wq):
            cnt = np.bincount(wq)
            TC_ = max(TC_, int((cnt.max() + P - 1) // P))

    SLOTW = 4 * TC_ * P
    T_tot = WPC * 4 * TC_
    L = T_tot * P

    per_core = []
    meta_cores = []
    for k in range(n_cores):
        e0, order, w_of, q_of = groups[k]
        nlo, nhi = node_lo[k], node_hi[k]
        dloc = np.full((WPC, 4, TC_ * P), -1.0, dtype=np.float32)
        eaw = np.zeros((WPC, 4, TC_ * P, edim), dtype=np.float16)
        sidx = np.zeros((WPC, 4, TC_ * P), dtype=np.int16)
        oe = e0 + order
        wi = w_of[order]
        qi = q_of[order]
        wq = wi * 4 + qi
        pos = np.zeros(len(order), dtype=np.int64)
        if len(order):
            brk = np.flatnonzero(np.diff(wq)) + 1
            starts = np.concatenate([[0], brk])
            lens = np.diff(np.concatenate([starts, [len(order)]]))
            pos = np.arange(len(order)) - np.repeat(starts, lens)
        dloc[wi, qi, pos] = (d_s[oe] - nlo - wi * P).astype(np.float32)
        eaw[wi, qi, pos] = ea_s[oe].astype(np.float16)
        sidx[wi, qi, pos] = (s_s[oe] - qi * CH).astype(np.int16)

        # gather stream: chunk-major [q][b][wb windows] wrapped mod 16
        sidx_cm = np.transpose(sidx.reshape(WPC, 4, TC_ * P), (1, 0, 2))
        nbs = WPC // wb
        bsz = wb * TC_ * P
        sidx_b = sidx_cm.reshape(4, nbs, bsz)
        sidx_w = np.transpose(
            sidx_b.reshape(4, nbs, bsz // 16, 16), (0, 1, 3, 2)
        )
        sidx_w = np.ascontiguousarray(
            np.transpose(sidx_w, (2, 0, 1, 3)).reshape(16, 4 * nbs * (bsz // 16))
        )
        sidx_w = np.tile(sidx_w, (8, 1))  # replicate for the 8 Q7 cores

        dloc_pm = np.ascontiguousarray(dloc.reshape(T_tot, P).T)
        dlocF = dloc.reshape(1, L).astype(np.float16)
        eaT = np.ascontiguousarray(eaw.reshape(L, edim).T)

        xT_loc = np.zeros((x.shape[1], WPC * P), dtype=np.float16)
        n_take = min(WPC * P, N - nlo)
        xT_loc[:, :n_take] = x[nlo : nlo + n_take].T.astype(np.float16)

        per_core.append(
            dict(sidx16=sidx_w, dloc=dloc_pm, dlocF=dlocF, eaT=eaT,
                 xT_loc=xT_loc)
        )
        meta_cores.append(dict(nlo=nlo, nhi=nhi))

    meta = dict(
        N=N, E=E, H=heads, D=head_dim, edim=edim, n_cores=n_cores,
        NPAD=NPAD, CH=CH, WPC=WPC, TC_=TC_, T_tot=T_tot, L=L, wb=wb,
        cores=meta_cores, per_core=per_core,
    )
    return meta


def host_unscramble(meta, results, out_dim, dtype):
    N = meta["N"]
    out = np.zeros((N, out_dim), dtype=dtype)
    for k, c in enumerate(meta["cores"]):
        op = results[k]["out_pad"]
        nlo, nhi = c["nlo"], c["nhi"]
        nw = (nhi - nlo + P - 1) // P
        for w in range(nw):
            lo = nlo + w * P
            sp = min(P, nhi - lo)
            out[lo : lo + sp] = op[w * P : w * P + sp]
    return out


# ---------------------------------------------------------------- kernel

def build_nc(meta, in_dim, out_dim, c_shift=6.0, eps=1e-9):
    H = meta["H"]
    D = meta["D"]
    edim = meta["edim"]
    WPC = meta["WPC"]
    TC_ = meta["TC_"]
    T_tot = meta["T_tot"]
    L = meta["L"]
    CH = meta["CH"]
    NPAD = meta["NPAD"]
    wb = meta["wb"]
    OD = out_dim
    TC = OD + H  # 68
    TW = 4 * TC_
    n_rt = NPAD // P
    nbs = WPC // wb
    bsz = wb * TC_ * P
    btiles = wb * TC_

    nc = bacc.Bacc()
    f16, f32, i16, i32 = DT.float16, DT.float32, DT.int16, DT.int32

    xT = nc.dram_tensor("xT", [in_dim, NPAD], f16, kind="ExternalInput").ap()
    xT_loc = nc.dram_tensor("xT_loc", [in_dim, WPC * P], f16, kind="ExternalInput").ap()
    w_nodeT = nc.dram_tensor("w_nodeT", [in_dim, OD], f32, kind="ExternalInput").ap()
    att_src_b = nc.dram_tensor("att_src_b", [P, OD], f32, kind="ExternalInput").ap()
    att_dst_b = nc.dram_tensor("att_dst_b", [P, OD], f32, kind="ExternalInput").ap()
    w_edgeT = nc.dram_tensor("w_edgeT", [edim, H], f16, kind="ExternalInput").ap()
    bias_b = nc.dram_tensor("bias_b", [P, OD], f32, kind="ExternalInput").ap()
    iota_d = nc.dram_tensor("iota", [P, P], f16, kind="ExternalInput").ap()
    iotaP_d = nc.dram_tensor("iotaP", [P, 1], f32, kind="ExternalInput").ap()
    ones_d = nc.dram_tensor("ones1", [1, P], f16, kind="ExternalInput").ap()
    sidx_d = nc.dram_tensor("sidx16", [P, T_tot * 8], i16, kind="ExternalInput").ap()
    dloc_d = nc.dram_tensor("dloc", [P, T_tot], f32, kind="ExternalInput").ap()
    dlocF_d = nc.dram_tensor("dlocF", [1, L], f16, kind="ExternalInput").ap()
    eaT_d = nc.dram_tensor("eaT", [edim, L], f16, kind="ExternalInput").ap()

    out_pad = nc.dram_tensor("out_pad", [WPC * P, OD], f32, kind="ExternalOutput").ap()

    T1 = nc.dram_tensor("T1", [NPAD, P], f16).ap()

    with tile.TileContext(nc) as tc:
        with tc.tile_pool(name="const", bufs=1) as cpool:
            iota_sb = cpool.tile([P, P], f16)
            nc.sync.dma_start(out=iota_sb[:], in_=iota_d[:])
            iotaP_sb = cpool.tile([P, 1], f32)
            nc.sync.dma_start(out=iotaP_sb[:], in_=iotaP_d[:])
            ones_sb = cpool.tile([1, P], f16)
            nc.sync.dma_start(out=ones_sb[:], in_=ones_d[:])
            negc_sb = cpool.tile([P, 1], f32)
            nc.vector.memset(negc_sb[:], -c_shift)
            wE_sb = cpool.tile([edim, H], f16)
            nc.sync.dma_start(out=wE_sb[:], in_=w_edgeT[:])
            bias_sb = cpool.tile([P, OD], f32)
            nc.sync.dma_start(out=bias_sb[:], in_=bias_b[:])
            t2sb = cpool.tile([P, WPC * H], f16)
            t2v = t2sb[:].rearrange("p (w h) -> p w h", h=H)

            # ---------------- phase 1: node table
            with (
                tc.tile_pool(name="p1c", bufs=1) as c1,
                tc.tile_pool(name="p1", bufs=3) as p1,
                tc.tile_pool(name="p1ps", bufs=2, space="PSUM") as p1ps,
            ):
                wN_sb = c1.tile([in_dim, OD], f32)
                nc.sync.dma_start(out=wN_sb[:], in_=w_nodeT[:])
                aS_sb = c1.tile([P, OD], f32)
                nc.sync.dma_start(out=aS_sb[:], in_=att_src_b[:])
                aD_sb = c1.tile([P, OD], f32)
                nc.sync.dma_start(out=aD_sb[:], in_=att_dst_b[:])

                B_sb = c1.tile([in_dim, P], f16)
                nc.vector.memset(B_sb[:], 0.0)
                nc.vector.tensor_copy(B_sb[:, 0:OD], wN_sb[:])
                prod = c1.tile([in_dim, OD], f32)
                red = c1.tile([in_dim, 2 * H], f32)
                nc.vector.tensor_tensor(
                    out=prod[:], in0=wN_sb[:], in1=aS_sb[:in_dim], op=ALU.mult
                )
                for h in range(H):
                    nc.vector.reduce_sum(
                        red[:, h : h + 1], prod[:, h * D : (h + 1) * D],
                        axis=mybir.AxisListType.X,
                    )
                nc.vector.tensor_tensor(
                    out=prod[:], in0=wN_sb[:], in1=aD_sb[:in_dim], op=ALU.mult
                )
                for h in range(H):
                    nc.vector.reduce_sum(
                        red[:, H + h : H + h + 1], prod[:, h * D : (h + 1) * D],
                        axis=mybir.AxisListType.X,
                    )
                nc.vector.tensor_copy(B_sb[:, OD : OD + 2 * H], red[:])

                for r in range(n_rt):
                    xt = p1.tile([in_dim, P], f16, tag="xt")
                    nc.sync.dma_start(out=xt[:], in_=xT[:, r * P : (r + 1) * P])
                    ps = p1ps.tile([P, P], f32)
                    nc.tensor.matmul(ps[:], lhsT=xt[:], rhs=B_sb[:],
                                     start=True, stop=True)
                    t1sb = p1.tile([P, P], f16, tag="t1sb")
                    nc.scalar.copy(t1sb[:], ps[:])
                    nc.sync.dma_start(out=T1[r * P : (r + 1) * P, :], in_=t1sb[:])

                # phase 1b: local a_dst windows -> t2sb
                for w in range(WPC):
                    xt = p1.tile([in_dim, P], f16, tag="xt")
                    nc.sync.dma_start(out=xt[:], in_=xT_loc[:, w * P : (w + 1) * P])
                    ps = p1ps.tile([P, P], f32)
                    nc.tensor.matmul(ps[:], lhsT=xt[:], rhs=B_sb[:],
                                     start=True, stop=True)
                    nc.scalar.copy(t2v[:, w, :], ps[:, TC : TC + H])

            # ---------------- phase 2
            with (
                tc.tile_pool(name="acc2", bufs=1) as apool2,
                tc.tile_pool(name="g", bufs=2) as gp,
                tc.tile_pool(name="wk", bufs=3) as wk,
                tc.tile_pool(name="oh", bufs=3) as ohp,
                tc.tile_pool(name="ps2", bufs=2, space="PSUM") as ps2,
                tc.tile_pool(name="ps3", bufs=2, space="PSUM") as ps3,
                tc.tile_pool(name="psr", bufs=2, space="PSUM") as psr,
                tc.tile_pool(name="o3", bufs=3) as o3p,
            ):
                acc2 = apool2.tile([P, WPC * TC], f32)
                nc.vector.memset(acc2[:], 0.0)
                acc2v = acc2[:].rearrange("p (w c) -> p w c", c=TC)

                for b in range(nbs):
                    gsq = []
                    for q in range(4):
                        g = gp.tile([P, btiles * P], f16, tag=f"gs{q}")
                        ixq = gp.tile([P, bsz // 16], i16, tag=f"ix{q}")
                        o = (q * nbs + b) * (bsz // 16)
                        nc.sync.dma_start(
                            out=ixq[:], in_=sidx_d[:, o : o + bsz // 16]
                        )
                        nc.gpsimd.dma_gather(
                            out_ap=g[:].rearrange("p (t c) -> p t c", c=P),
                            in_ap=T1[q * CH : (q + 1) * CH, :],
                            idxs_ap=ixq[:],
                            num_idxs=bsz,
                            num_idxs_reg=bsz,
                            elem_size=P,
                            single_packet=False,
                        )
                        gsq.append(g[:].rearrange("p (t c) -> p t c", c=P))
                    t0 = b * wb * TW
                    dlc = gp.tile([P, wb * TW], f32, tag="dlc")
                    nc.sync.dma_start(out=dlc[:], in_=dloc_d[:, t0 : t0 + wb * TW])

                    for wi in range(wb):
                        w = b * wb + wi
                        tw0 = (w * TW) * P
                        dlf = wk.tile([1, TW * P], f16, tag="dlf")
                        nc.sync.dma_start(
                            out=dlf[:], in_=dlocF_d[:, tw0 : tw0 + TW * P]
                        )
                        eat = wk.tile([edim, TW * P], f16, tag="eat")
                        nc.sync.dma_start(
                            out=eat[:], in_=eaT_d[:, tw0 : tw0 + TW * P]
                        )
                        psc = ps3.tile([P, TW * H], f32)
                        for tt in range(TW):
                            tl = wi * TW + tt
                            nc.tensor.matmul(
                                psc[:, tt * H : (tt + 1) * H],
                                lhsT=eat[:, tt * P : (tt + 1) * P],
                                rhs=wE_sb[:], start=True, stop=False,
                            )
                            rp = psr.tile([P, P], f32)
                            nc.tensor.matmul(
                                rp[:], lhsT=ones_sb[:],
                                rhs=dlf[:, tt * P : (tt + 1) * P],
                                start=True, stop=True,
                            )
                            ohT = ohp.tile([P, P], f16, tag="ohT")
                            nc.vector.tensor_scalar(
                                out=ohT[:], in0=rp[:], scalar1=iotaP_sb[:],
                                scalar2=None, op0=ALU.is_equal,
                            )
                            nc.tensor.matmul(
                                psc[:, tt * H : (tt + 1) * H],
                                lhsT=ohT[:], rhs=t2v[:, w, :],
                                start=False, stop=True,
                            )
                        e16 = wk.tile([P, TW * H], f16, tag="e16")
                        e16v = e16[:].rearrange("p (t h) -> p t h", h=H)
                        pscv = psc[:].rearrange("p (t h) -> p t h", h=H)
                        for q in range(4):
                            gq = gsq[q]
                            tb0 = wi * TC_
                            nc.vector.tensor_tensor(
                                out=e16v[:, q * TC_ : (q + 1) * TC_, :],
                                in0=pscv[:, q * TC_ : (q + 1) * TC_, :],
                                in1=gq[:, tb0 : tb0 + TC_, OD:TC],
                                op=ALU.add,
                            )
                        nc.vector.scalar_tensor_tensor(
                            out=e16[:], in0=e16[:], scalar=0.2, in1=e16[:],
                            op0=ALU.mult, op1=ALU.max,
                        )
                        ex = wk.tile([P, TW * H], f16, tag="ex")
                        nc.scalar.activation(ex[:], e16[:], AF.Exp, bias=negc_sb[:])

                        pagg = ps2.tile([P, TC], f32)
                        for tt in range(TW):
                            q, k = tt // TC_, tt % TC_
                            tl = wi * TW + tt
                            oh = ohp.tile([P, P], f16, tag="oh")
                            nc.vector.tensor_scalar(
                                out=oh[:], in0=iota_sb[:],
                                scalar1=dlc[:, tl : tl + 1], scalar2=None,
                                op0=ALU.is_equal,
                            )
                            mg = ohp.tile([P, TC], f16, tag="mg")
                            exs = ex[:, tt * H : (tt + 1) * H]
                            gq = gsq[q]
                            tb = wi * TC_ + k
                            nc.vector.tensor_tensor(
                                out=mg[:, 0:OD].rearrange("p (h e) -> p h e", h=H),
                                in0=gq[:, tb, 0:OD].rearrange("p (h e) -> p h e", h=H),
                                in1=exs.unsqueeze(2).to_broadcast([P, H, D]),
                                op=ALU.mult,
                            )
                            nc.scalar.copy(mg[:, OD:TC], exs)
                            nc.tensor.matmul(
                                pagg[:], lhsT=oh[:], rhs=mg[:],
                                start=(tt == 0), stop=(tt == TW - 1),
                            )
                        nc.vector.tensor_tensor(
                            out=acc2v[:, w, :], in0=acc2v[:, w, :],
                            in1=pagg[:], op=ALU.add,
                        )

                # ---------------- phase 3
                for g in range(WPC):
                    den = o3p.tile([P, H], f32, tag="den")
                    nc.vector.tensor_scalar(
                        out=den[:], in0=acc2v[:, g, OD:TC], scalar1=eps,
                        scalar2=None, op0=ALU.add,
                    )
                    rc = o3p.tile([P, H], f32, tag="rc")
                    nc.vector.reciprocal(rc[:], den[:])
                    o1 = o3p.tile([P, OD], f32, tag="o1")
                    nc.vector.tensor_tensor(
                        out=o1[:].rearrange("p (h e) -> p h e", h=H),
                        in0=acc2v[:, g, 0:OD].rearrange("p (h e) -> p h e", h=H),
                        in1=rc[:].unsqueeze(2).to_broadcast([P, H, D]),
                        op=ALU.mult,
                    )
                    nc.vector.tensor_tensor(
                        out=o1[:], in0=o1[:], in1=bias_sb[:], op=ALU.add
                    )
                    t_ = o3p.tile([P, OD], f32, tag="t_")
                    nc.vector.tensor_scalar(
                        out=t_[:], in0=o1[:], scalar1=0.0, scalar2=None,
                        op0=ALU.min,
                    )
                    nc.scalar.activation(t_[:], t_[:], AF.Exp)
                    nc.vector.tensor_scalar(
                        out=t_[:], in0=t_[:], scalar1=-1.0, scalar2=None,
                        op0=ALU.add,
                    )
                    o2 = o3p.tile([P, OD], f32, tag="o2")
                    nc.vector.tensor_tensor(
                        out=o2[:], in0=o1[:], in1=t_[:], op=ALU.max
                    )
                    nc.sync.dma_start(
                        out=out_pad[g * P : (g + 1) * P, :], in_=o2[:]
                    )

    nc.compile()
    return nc


# ---------------------------------------------------------------- driver

def run_gat(x, ei, ea, W_node, W_edge, att_src, att_dst, bias,
            n_cores=8, wb=5, c_shift=6.0, sim=False, trace=False, **kw):
    N, in_dim = x.shape
    out_dim = W_node.shape[0]
    H = W_edge.shape[0]
    D = out_dim // H
    meta = host_prep(x, ei, ea, n_cores, H, D, wb=wb)

    NPAD = meta["NPAD"]
    xTp = np.zeros((in_dim, NPAD), dtype=np.float16)
    xTp[:, :N] = x.T.astype(np.float16)

    shared = dict(
        xT=xTp,
        w_nodeT=np.ascontiguousarray(W_node.T.astype(np.float32)),
        att_src_b=np.tile(att_src.reshape(1, out_dim), (P, 1)).astype(np.float32),
        att_dst_b=np.tile(att_dst.reshape(1, out_dim), (P, 1)).astype(np.float32),
        w_edgeT=np.ascontiguousarray(W_edge.T.astype(np.float16)),
        bias_b=np.tile(bias.reshape(1, out_dim), (P, 1)).astype(np.float32),
        iota=np.tile(np.arange(P, dtype=np.float16).reshape(1, P), (P, 1)),
        iotaP=np.arange(P, dtype=np.float32).reshape(P, 1),
        ones1=np.ones((1, P), dtype=np.float16),
    )
    in_maps = []
    for k in range(n_cores):
        m = dict(shared)
        m.update(meta["per_core"][k])
        in_maps.append(m)

    nc = build_nc(meta, in_dim, out_dim, c_shift=c_shift)

    if sim:
        from concourse import bass_interp
        msim = bass_interp.MultiCoreSim(nc, n_cores)
        for k in range(n_cores):
            for name, arr in in_maps[k].items():
                msim.cores[k].tensor(name)[:] = arr
        msim.simulate()
        results = [
            {"out_pad": np.array(msim.cores[k].tensor("out_pad"))}
            for k in range(n_cores)
        ]
        return host_unscramble(meta, results, out_dim, np.float32), None

    res = run_bass_kernel_spmd(nc, in_maps, list(range(n_cores)), trace=trace)
    out = host_unscramble(meta, res.results, out_dim, np.float32)
    return out, res


# ---------------------------------------------------------------- entry point

def kernel(x, ei, ea, W_node, W_edge, att_src, att_dst, bias):
    """Full-input GAT layer on 8 trn2 NeuronCores. Returns [N, 64] float32."""
    x = np.asarray(x, dtype=np.float32)
    ei = np.asarray(ei, dtype=np.int32)
    ea = np.asarray(ea, dtype=np.float32)
    W_node = np.asarray(W_node, dtype=np.float32)
    W_edge = np.asarray(W_edge, dtype=np.float32)
    att_src = np.asarray(att_src, dtype=np.float32)
    att_dst = np.asarray(att_dst, dtype=np.float32)
    bias = np.asarray(bias, dtype=np.float32)
    out, _ = run_gat(x, ei, ea, W_node, W_edge, att_src, att_dst, bias,
                     n_cores=8)
    return out

